# revision 12
# baseline (speedup 1.0000x reference)
"""AdaptiveGlobalWeightedRankPooling2d on 8 Trainium2 NeuronCores.

Math: y[b,c] = sum_n sort_desc(x[b,c])[n] * w[c,n] / sum_n w[c,n]
with w[c,n] = sigmoid(dc_logit[c] ** n).  In f32, w[c,n] == 0.5 exactly
for n >= 18 (dc_logit ~ 0.4055); ranks >= 8 deviate by < 2e-4 in total
weight, so

    y[b,c] = sum_{j<8} top_j * wu[c,j]  +  S[b,c] * (0.5 / sum_w[c])

with wu[c,j] = (w[c,j]-0.5)/sum_w[c] host-precomputed and S the full row
sum.  x is staged to the device as bf16 (rel-err floor ~1.7e-3, gate
2e-2; host-validated pipeline ~2.0e-3).

Per core: 1024 rows of N=16384 bf16 = 8 tiles x [128, 16384], DMAed as
16 2MB segments into an 8-slot SBUF ring.

Engine split (measured op costs):
  PE (idle otherwise) computes ALL row sums: identity-stationary
    matmuls accumulate 32 x [128, 512] chunks per tile into a PSUM bank
    (A[p,f] += x[p, 512c+f]; 216ns/chunk steady state, ~55us total).
    Numerically exact f32 accumulation of the bf16 values.
  ScalarE reduces each tile's [128, 512] PSUM bank with one
    activation-accumulate into rall (~0.9us/tile) - else idle.
  DVE owns the max path: per tile pair, f1_a/f1_b (8192-out 2x
    tensor_max), merged f2ab/f3ab/f4ab folds -> [2, 1024], then max8
    (sorted top-8) per tile; end: cast top8s->f32, 80-wide mul by host
    weights, grouped reduce -> out [128, 8].  ~80us busy = critical
    engine.  The last pair is split per-tile with seg14/15 DMAed in
    halves so only ~8us of DVE work remains after the final byte lands.

DVE write-ack pipelining: ops are spaced >= 2 after their producer where
the pair structure allows; remaining one-back consumers carry a serial
vchain wait.
"""

import numpy as np

B, C, H, W = 32, 256, 128, 128
N = H * W                 # 16384
NCORES = 8
BS = B // NCORES          # 4 batches per core
ROWS = BS * C             # 1024 rows per core
P = 128                   # partitions
NTILES = ROWS // P        # 8
SEG = 8192                # bf16 elems per segment (2MB per [128, SEG] tile)
NSEG = N // SEG           # 2
NSEGS = NTILES * NSEG     # 16
NSLOT = 9                 # SBUF ring depth
K = 8                     # top-K kept
RW = 10                   # rall cols per tile: 8 top + 1 sum + 1 pad
CH = 512                  # PE chunk width (one PSUM bank)
NCH = N // CH             # 32 chunks per tile
NBANK = 4                 # PSUM banks in rotation

_CACHE = {}


def _schedule():
    """Static DVE op order; 'ser'=1 marks one-back (serial) consumers."""
    ops = []
    ops += [("f1q0", 0, 0), ("f1q1", 0, 0), ("f1R0", 0, 0), ("f1", 1, 0),
            ("f2ab", 0, 1), ("f3ab", 0, 1), ("f4ab", 0, 1),
            ("m8", 0, 1), ("m8", 1, 0)]
    for k in (1, 2):
        a, b = 2 * k, 2 * k + 1
        ops += [("f1", a, 0), ("f1", b, 0), ("f2ab", a, 1), ("f3ab", a, 1),
                ("f4ab", a, 1), ("m8", a, 1), ("m8", b, 0)]
    # last pair: tiles 6/7 interleaved, tile 7 folds per DMA-half so only
    # the last half-segment's chain (~6.5us) remains after the final byte
    ops += [("f1", 6, 0), ("t7B", 7, 0), ("f2s", 6, 0), ("t7A", 7, 0),
            ("f3s", 6, 0), ("t7E", 7, 0), ("f4s", 6, 0), ("m8s", 6, 1),
            ("t7C", 7, 0), ("t7D", 7, 1), ("t7F", 7, 1), ("t7G", 7, 1),
            ("m8s", 7, 1)]
    ops += [("cast", -1, 1), ("mul", -1, 1), ("red", -1, 1)]
    pos = {}
    for i, op in enumerate(ops):
        pos[(op[0], op[1])] = i + 1
    return ops, pos


def _build():
    if "nc_raw" in _CACHE:
        return _CACHE["nc_raw"]
    from concourse import bacc, mybir

    f32 = mybir.dt.float32
    bf16 = mybir.dt.bfloat16
    Copy = mybir.ActivationFunctionType.Copy
    X = mybir.AxisListType.X
    add = mybir.AluOpType.add

    ops, pos = _schedule()
    V_TOTAL = len(ops)
    A_TOTAL = NTILES  # one PSUM-reduce ACT per tile

    # ring frees: seg j (< 8) must be released for seg j+8 by
    #  - DVE: its f1 read      - PE: its 16 chunk matmuls
    v_free = {}
    for t in range(4):          # tiles 0..3 own segs 0..7
        sL, sR = 2 * t, 2 * t + 1
        if t == 0:
            v_free[sL] = pos[("f1q1", 0)]
            v_free[sR] = pos[("f1R0", 0)]
        else:
            v_free[sL] = pos[("f1", t)]
            v_free[sR] = pos[("f1", t)]
    p_free = {j: 32 * (j // 2) + 16 * (j % 2 + 1) for j in range(NSEGS)}

    nc = bacc.Bacc(
        "TRN2", target_bir_lowering=False, debug=False, num_devices=NCORES
    )
    x = nc.dram_tensor("x", [ROWS, N], bf16, kind="ExternalInput").ap()
    cpk = nc.dram_tensor("cpk", [P, NTILES * RW], f32, kind="ExternalInput").ap()
    ident = nc.dram_tensor("ident", [P, P], bf16, kind="ExternalInput").ap()
    out = nc.dram_tensor("out", [P, NTILES], f32, kind="ExternalOutput").ap()
    import os
    dbg = bool(os.environ.get("KERNEL_DEBUG_RALL"))
    rall_out = (
        nc.dram_tensor("rall_out", [P, NTILES * RW], f32, kind="ExternalOutput").ap()
        if dbg
        else None
    )

    xbuf = nc.alloc_sbuf_tensor("xbuf", [P, NSLOT * SEG], bf16).ap()
    m1 = nc.alloc_sbuf_tensor("m1", [P, 2 * SEG], bf16).ap()   # [a | b]
    f2ab = nc.alloc_sbuf_tensor("f2ab", [P, SEG], bf16).ap()   # [2, 4096]
    f3ab = nc.alloc_sbuf_tensor("f3ab", [P, SEG // 2], bf16).ap()
    f4ab = nc.alloc_sbuf_tensor("f4ab", [P, SEG // 4], bf16).ap()
    idsb = nc.alloc_sbuf_tensor("idsb", [P, P], bf16).ap()
    dumact = nc.alloc_sbuf_tensor("dumact", [P, CH], bf16).ap()
    cand = nc.alloc_sbuf_tensor("cand", [P, NTILES * K], bf16).ap()
    rall = nc.alloc_sbuf_tensor("rall", [P, NTILES * RW], f32).ap()
    scr = nc.alloc_sbuf_tensor("scr", [P, NTILES * RW], f32).ap()
    cpksb = nc.alloc_sbuf_tensor("cpksb", [P, NTILES * RW], f32).ap()
    outsb = nc.alloc_sbuf_tensor("outsb", [P, NTILES], f32).ap()

    banks = [nc.alloc_psum_tensor(f"acc{i}", [P, CH], f32).ap() for i in range(NBANK)]

    seg_sem = [nc.alloc_semaphore(f"seg{k}") for k in range(NSLOT)]
    seg0a_sem = nc.alloc_semaphore("seg0a")
    s14a_sem = nc.alloc_semaphore("s14a")
    s15a_sem = nc.alloc_semaphore("s15a")
    cst_sem = nc.alloc_semaphore("cst")
    mset_sem = nc.alloc_semaphore("mset")
    out_sem = nc.alloc_semaphore("outd")
    vchain = nc.alloc_semaphore("vchain")
    achain = nc.alloc_semaphore("achain")
    pe_sem = nc.alloc_semaphore("pe_sem")

    def seg_thresh(i):
        return 16 * (i // NSLOT + 1)

    def slot(i):
        return xbuf[:, (i % NSLOT) * SEG : (i % NSLOT + 1) * SEG]

    with nc.Block(no_gpsimd_drain=True) as block:

        def issue_seg(eng, i):
            if i >= NSLOT:
                j = i - NSLOT
                if j in v_free:
                    eng.wait_ge(vchain, v_free[j])
                eng.wait_ge(pe_sem, p_free[j])
            t, sg = divmod(i, NSEG)
            src = x[t * P : (t + 1) * P, sg * SEG : (sg + 1) * SEG]
            if i in (0, 14, 15):
                half_sem = {0: seg0a_sem, 14: s14a_sem, 15: s15a_sem}[i]
                eng.dma_start(
                    out=slot(i)[:, 0 : SEG // 2], in_=src[:, 0 : SEG // 2]
                ).then_inc(half_sem, 16)
                eng.dma_start(
                    out=slot(i)[:, SEG // 2 : SEG], in_=src[:, SEG // 2 : SEG]
                ).then_inc(seg_sem[i % NSLOT], 16)
            else:
                eng.dma_start(out=slot(i), in_=src).then_inc(
                    seg_sem[i % NSLOT], 16
                )

        @block.sync
        def _(sync):
            for i in range(0, NSEGS, 2):
                issue_seg(sync, i)
            sync.wait_ge(vchain, V_TOTAL)
            sync.dma_start(out=out[:], in_=outsb[:]).then_inc(out_sem, 16)
            if dbg:
                sync.dma_start(out=rall_out[:], in_=rall[:]).then_inc(out_sem, 16)
            sync.wait_ge(out_sem, 32 if dbg else 16)

        @block.gpsimd
        def _(g):
            g.memset(rall[:], 0.0).then_inc(mset_sem, 1)
            for i in range(1, NSEGS, 2):
                issue_seg(g, i)

        @block.tensor
        def _(te):
            te.wait_ge(cst_sem, 32)  # idsb loaded (scalar dma incs by 32)
            for t in range(NTILES):
                bank = banks[t % NBANK]
                if t >= NBANK:
                    te.wait_ge(achain, t - NBANK + 1)
                for c in range(NCH):
                    sg = 2 * t + c // 16
                    if c == 0:
                        if sg == 0:
                            te.wait_ge(seg0a_sem, 16)
                        elif sg == 14:
                            te.wait_ge(s14a_sem, 16)
                        else:
                            te.wait_ge(seg_sem[sg % NSLOT], seg_thresh(sg))
                    elif c == 8 and sg == 0:
                        te.wait_ge(seg_sem[0], 16)
                    elif c == 8 and sg == 14:
                        te.wait_ge(seg_sem[14 % NSLOT], seg_thresh(14))
                    elif c == 16:
                        if sg == 15:
                            te.wait_ge(s15a_sem, 16)
                        else:
                            te.wait_ge(seg_sem[sg % NSLOT], seg_thresh(sg))
                    elif c == 24 and sg == 15:
                        te.wait_ge(seg_sem[15 % NSLOT], seg_thresh(15))
                    off = (c % 16) * CH
                    te.matmul(
                        bank[:],
                        idsb[:],
                        slot(sg)[:, off : off + CH],
                        start=(c == 0),
                        stop=(c == NCH - 1),
                    ).then_inc(pe_sem, 1)

        @block.scalar
        def _(s):
            s.dma_start(out=cpksb[:], in_=cpk[:]).then_inc(cst_sem, 16)
            s.dma_start(out=idsb[:], in_=ident[:]).then_inc(cst_sem, 16)
            s.wait_ge(mset_sem, 1)
            for t in range(NTILES):
                s.wait_ge(pe_sem, 32 * (t + 1))
                ins = s.activation(
                    dumact[:],
                    banks[t % NBANK][:],
                    Copy,
                    bias=0.0,
                    scale=1.0,
                    accum_out=rall[:, t * RW + K : t * RW + K + 1],
                )
                if t >= 2:
                    ins._wait_ge(achain, t - 1)
                ins.then_inc(achain)

        @block.vector
        def _(v):
            vc = 0

            def emit(ins, serial=False):
                nonlocal vc
                ins._wait_ge(vchain, vc if serial else max(0, vc - 1))
                ins.then_inc(vchain)
                vc += 1

            v.wait_ge(cst_sem, 32)
            v.wait_ge(mset_sem, 1)

            for kind, t, ser in ops:
                h = t % 2
                mt = m1[:, h * SEG : (h + 1) * SEG]
                if kind == "f1q0":
                    v.wait_ge(seg0a_sem, 16)
                    emit(v.tensor_max(
                        m1[:, 0 : SEG // 4],
                        xbuf[:, 0 : SEG // 4],
                        xbuf[:, SEG // 4 : SEG // 2],
                    ))
                elif kind == "f1q1":
                    v.wait_ge(seg_sem[0], 16)
                    emit(v.tensor_max(
                        m1[:, SEG // 4 : SEG // 2],
                        xbuf[:, SEG // 2 : SEG // 2 + SEG // 4],
                        xbuf[:, SEG // 2 + SEG // 4 : SEG],
                    ))
                elif kind == "f1R0":
                    v.wait_ge(seg_sem[1], 16)
                    emit(v.tensor_max(
                        m1[:, SEG // 2 : SEG],
                        xbuf[:, SEG : SEG + SEG // 2],
                        xbuf[:, SEG + SEG // 2 : 2 * SEG],
                    ))
                elif kind == "f1":
                    sL, sR = 2 * t, 2 * t + 1
                    v.wait_ge(seg_sem[sL % NSLOT], seg_thresh(sL))
                    v.wait_ge(seg_sem[sR % NSLOT], seg_thresh(sR))
                    emit(v.tensor_max(mt, slot(sL), slot(sR)))
                elif kind == "t7A":
                    # fold within seg14 -> m1b[0:4096]
                    v.wait_ge(s14a_sem, 16)
                    v.wait_ge(seg_sem[14 % NSLOT], seg_thresh(14))
                    emit(v.tensor_max(
                        mt[:, 0 : SEG // 2],
                        slot(14)[:, 0 : SEG // 2],
                        slot(14)[:, SEG // 2 : SEG],
                    ))
                elif kind == "t7B":
                    # fold within seg15 first half -> m1b[4096:6144]
                    v.wait_ge(s15a_sem, 16)
                    emit(v.tensor_max(
                        mt[:, SEG // 2 : SEG // 2 + SEG // 4],
                        slot(15)[:, 0 : SEG // 4],
                        slot(15)[:, SEG // 4 : SEG // 2],
                    ))
                elif kind == "t7C":
                    # fold within seg15 second half -> m1b[6144:8192]
                    v.wait_ge(seg_sem[15 % NSLOT], seg_thresh(15))
                    emit(v.tensor_max(
                        mt[:, SEG // 2 + SEG // 4 : SEG],
                        slot(15)[:, SEG // 2 : SEG // 2 + SEG // 4],
                        slot(15)[:, SEG // 2 + SEG // 4 : SEG],
                    ))
                elif kind == "t7E":
                    # E = max(fold14.lo, fold14.hi) -> f2ab[4096:6144]
                    emit(v.tensor_max(
                        f2ab[:, SEG // 2 : SEG // 2 + SEG // 4],
                        mt[:, 0 : SEG // 4],
                        mt[:, SEG // 4 : SEG // 2],
                    ), serial=bool(ser))
                elif kind == "t7D":
                    # D = max(fold15lo, fold15hi) -> f2ab[6144:8192]
                    emit(v.tensor_max(
                        f2ab[:, SEG // 2 + SEG // 4 : SEG],
                        mt[:, SEG // 2 : SEG // 2 + SEG // 4],
                        mt[:, SEG // 2 + SEG // 4 : SEG],
                    ), serial=bool(ser))
                elif kind == "t7F":
                    # F = max(E, D) -> f3ab[2048:4096]
                    emit(v.tensor_max(
                        f3ab[:, SEG // 4 : SEG // 2],
                        f2ab[:, SEG // 2 : SEG // 2 + SEG // 4],
                        f2ab[:, SEG // 2 + SEG // 4 : SEG],
                    ), serial=bool(ser))
                elif kind == "t7G":
                    # G = fold(F) -> f4ab[1024:2048]
                    emit(v.tensor_max(
                        f4ab[:, SEG // 8 : SEG // 4],
                        f3ab[:, SEG // 4 : SEG // 4 + SEG // 8],
                        f3ab[:, SEG // 4 + SEG // 8 : SEG // 2],
                    ), serial=bool(ser))
                elif kind == "f2ab":
                    emit(v.tensor_max(
                        f2ab.rearrange("p (g w) -> p g w", w=SEG // 2),
                        m1.rearrange("p (g w) -> p g w", w=SEG)[:, :, 0 : SEG // 2],
                        m1.rearrange("p (g w) -> p g w", w=SEG)[:, :, SEG // 2 : SEG],
                    ), serial=bool(ser))
                elif kind == "f3ab":
                    emit(v.tensor_max(
                        f3ab.rearrange("p (g w) -> p g w", w=SEG // 4),
                        f2ab.rearrange("p (g w) -> p g w", w=SEG // 2)[:, :, 0 : SEG // 4],
                        f2ab.rearrange("p (g w) -> p g w", w=SEG // 2)[:, :, SEG // 4 : SEG // 2],
                    ), serial=bool(ser))
                elif kind == "f4ab":
                    emit(v.tensor_max(
                        f4ab.rearrange("p (g w) -> p g w", w=SEG // 8),
                        f3ab.rearrange("p (g w) -> p g w", w=SEG // 4)[:, :, 0 : SEG // 8],
                        f3ab.rearrange("p (g w) -> p g w", w=SEG // 4)[:, :, SEG // 8 : SEG // 4],
                    ), serial=bool(ser))
                elif kind == "m8":
                    emit(v.max(
                        cand[:, t * K : (t + 1) * K],
                        f4ab[:, h * (SEG // 8) : (h + 1) * (SEG // 8)],
                    ), serial=bool(ser))
                elif kind == "f2s":
                    emit(v.tensor_max(
                        f2ab[:, h * (SEG // 2) : (h + 1) * (SEG // 2)],
                        mt[:, 0 : SEG // 2], mt[:, SEG // 2 : SEG],
                    ), serial=bool(ser))
                elif kind == "f3s":
                    fs = f2ab[:, h * (SEG // 2) : (h + 1) * (SEG // 2)]
                    emit(v.tensor_max(
                        f3ab[:, h * (SEG // 4) : (h + 1) * (SEG // 4)],
                        fs[:, 0 : SEG // 4], fs[:, SEG // 4 : SEG // 2],
                    ), serial=bool(ser))
                elif kind == "f4s":
                    fs = f3ab[:, h * (SEG // 4) : (h + 1) * (SEG // 4)]
                    emit(v.tensor_max(
                        f4ab[:, h * (SEG // 8) : (h + 1) * (SEG // 8)],
                        fs[:, 0 : SEG // 8], fs[:, SEG // 8 : SEG // 4],
                    ), serial=bool(ser))
                elif kind == "m8s":
                    emit(v.max(
                        cand[:, t * K : (t + 1) * K],
                        f4ab[:, h * (SEG // 8) : (h + 1) * (SEG // 8)],
                    ), serial=bool(ser))
                elif kind == "cast":
                    emit(v.tensor_copy(
                        rall.rearrange("p (t r) -> p t r", r=RW)[:, :, 0:K],
                        cand.rearrange("p (t k) -> p t k", k=K),
                    ), serial=True)
                elif kind == "mul":
                    v.wait_ge(achain, A_TOTAL)
                    emit(v.tensor_mul(scr[:], rall[:], cpksb[:]), serial=True)
                elif kind == "red":
                    emit(v.tensor_reduce(
                        outsb[:],
                        scr.rearrange("p (t r) -> p t r", r=RW),
                        axis=X,
                        op=add,
                    ), serial=True)
            assert vc == V_TOTAL, (vc, V_TOTAL)

    nc.compile()
    _CACHE["nc_raw"] = nc
    return nc


def _host_weights(dc_logit: np.ndarray):
    """wu2[c, 0:8] = (w[c,j]-0.5)/sum_w[c]; col 8 = 0.5/sum_w; col 9 = 0."""
    dc = dc_logit.astype(np.float64)
    j = np.arange(N, dtype=np.float64)
    pw = dc[:, None] ** j[None, :]
    wfull = (1.0 / (1.0 + np.exp(-pw))).astype(np.float32)  # [C, N]
    dev = np.abs(wfull[:, K:] - np.float32(0.5))
    assert dev.max() < 2e-4, f"top-{K} decomposition invalid: {dev.max()}"
    sum_w = wfull.astype(np.float64).sum(axis=1)
    winv = 1.0 / sum_w
    wu2 = np.zeros((C, RW), np.float32)
    wu2[:, :K] = ((wfull[:, :K].astype(np.float64) - 0.5) * winv[:, None]).astype(
        np.float32
    )
    wu2[:, K] = (0.5 * winv).astype(np.float32)
    return wu2


def _run_pjrt(nc, in_maps):
    """Pre-uploads all inputs to the devices before dispatching the NEFF."""
    import jax
    import numpy as np
    from jax.sharding import Mesh, NamedSharding, PartitionSpec
    from jax.experimental.shard_map import shard_map
    from concourse import bass2jax, mybir

    bass2jax.install_neuronx_cc_hook()
    assert nc.dbg_addr is None
    n_cores = len(in_maps)
    partition_name = (
        nc.partition_id_tensor.name if nc.partition_id_tensor else None
    )

    in_names, out_names, out_avals, zero_outs = [], [], [], []
    for alloc in nc.m.functions[0].allocations:
        if not isinstance(alloc, mybir.MemoryLocationSet):
            continue
        name = alloc.memorylocations[0].name
        if alloc.kind == "ExternalInput":
            if name != partition_name:
                in_names.append(name)
        elif alloc.kind == "ExternalOutput":
            shape = tuple(alloc.tensor_shape)
            dtype = mybir.dt.np(alloc.dtype)
            out_names.append(name)
            out_avals.append(jax.core.ShapedArray(shape, dtype))
            zero_outs.append(np.zeros(shape, dtype))
    n_params = len(in_names)
    n_outs = len(out_avals)
    all_in_names = list(in_names) + out_names
    if partition_name is not None:
        all_in_names.append(partition_name)
    donate = tuple(range(n_params, n_params + n_outs))

    def _body(*args):
        operands = list(args)
        if partition_name is not None:
            operands.append(bass2jax.partition_id_tensor())
        return tuple(
            bass2jax._bass_exec_p.bind(
                *operands,
                out_avals=tuple(out_avals),
                in_names=tuple(all_in_names),
                out_names=tuple(out_names),
                lowering_input_output_aliases=(),
                sim_require_finite=True,
                sim_require_nnan=True,
                nc=nc,
            )
        )

    devices = jax.devices()[:n_cores]
    mesh = Mesh(np.asarray(devices), ("core",))
    spec = PartitionSpec("core")
    sharded = jax.jit(
        shard_map(
            _body,
            mesh=mesh,
            in_specs=(spec,) * (n_params + n_outs),
            out_specs=(spec,) * n_outs,
            check_rep=False,
        ),
        donate_argnums=donate,
        keep_unused=True,
    )
    sh = NamedSharding(mesh, spec)
    concat_in = [
        jax.device_put(
            np.concatenate([np.asarray(in_maps[c][k]) for c in range(n_cores)], axis=0),
            sh,
        )
        for k in in_names
    ]
    concat_zeros = [
        jax.device_put(
            np.zeros((n_cores * z.shape[0], *z.shape[1:]), z.dtype), sh
        )
        for z in zero_outs
    ]
    jax.block_until_ready(concat_in)
    jax.block_until_ready(concat_zeros)
    out_arrs = sharded(*concat_in, *concat_zeros)
    return [
        {
            name: np.asarray(out_arrs[i]).reshape(n_cores, *out_avals[i].shape)[c]
            for i, name in enumerate(out_names)
        }
        for c in range(n_cores)
    ]


def _in_maps(x: np.ndarray, dc_logit: np.ndarray):
    import ml_dtypes

    wu2 = _host_weights(np.asarray(dc_logit))  # [C, RW]
    cpk = np.empty((P, NTILES * RW), np.float32)
    for t in range(NTILES):
        cpk[:, t * RW : (t + 1) * RW] = wu2[(t % 2) * P : (t % 2 + 1) * P]
    ident = np.eye(P, dtype=np.float32).astype(ml_dtypes.bfloat16)
    xr = np.ascontiguousarray(x).reshape(B * C, N).astype(ml_dtypes.bfloat16)
    return [
        {"x": xr[i * ROWS : (i + 1) * ROWS], "cpk": cpk, "ident": ident}
        for i in range(NCORES)
    ]


def kernel(x: np.ndarray, dc_logit: np.ndarray) -> np.ndarray:
    import time

    nc = _build()
    in_maps = _in_maps(x, dc_logit)
    last_err = None
    for attempt in range(3):
        try:
            results = _run_pjrt(nc, in_maps)
            break
        except Exception as e:  # transient device errors (wedged core etc.)
            last_err = e
            time.sleep(15)
    else:
        raise last_err
    outs = []
    for i in range(NCORES):
        o = results[i]["out"]  # [P, NTILES]; col t, row p -> global row t*128+p
        outs.append(o.T.reshape(BS, C))
    return np.concatenate(outs, axis=0).astype(np.float32)


# revision 13
# speedup vs baseline: 1.1213x; 1.1213x over previous
"""AdaptiveGlobalWeightedRankPooling2d on 8 Trainium2 NeuronCores.

Math: y[b,c] = sum_n sort_desc(x[b,c])[n] * w[c,n] / sum_n w[c,n]
with w[c,n] = sigmoid(dc_logit[c] ** n).  In f32, w[c,n] == 0.5 exactly
for n >= 18 (dc_logit ~ 0.4055); ranks >= 8 deviate by < 2e-4 in total
weight, so

    y[b,c] = sum_{j<8} top_j * wu[c,j]  +  S[b,c] * (0.5 / sum_w[c])

with wu[c,j] = (w[c,j]-0.5)/sum_w[c] host-precomputed and S the full row
sum.  x is staged to the device as bf16 (rel-err floor ~1.7e-3, gate
2e-2; host-validated pipeline ~2.0e-3).

Per core: 1024 rows of N=16384 bf16 = 8 tiles x [128, 16384], DMAed as
16 2MB segments into an 8-slot SBUF ring.

Engine split (measured op costs):
  PE (idle otherwise) computes ALL row sums: identity-stationary
    matmuls accumulate 32 x [128, 512] chunks per tile into a PSUM bank
    (A[p,f] += x[p, 512c+f]; 216ns/chunk steady state, ~55us total).
    Numerically exact f32 accumulation of the bf16 values.
  ScalarE reduces each tile's [128, 512] PSUM bank with one
    activation-accumulate into rall (~0.9us/tile) - else idle.
  DVE owns the max path: per tile pair, f1_a/f1_b (8192-out 2x
    tensor_max), merged f2ab/f3ab/f4ab folds -> [2, 1024], then max8
    (sorted top-8) per tile; end: cast top8s->f32, 80-wide mul by host
    weights, grouped reduce -> out [128, 8].  ~80us busy = critical
    engine.  The last pair is split per-tile with seg14/15 DMAed in
    halves so only ~8us of DVE work remains after the final byte lands.

DVE write-ack pipelining: ops are spaced >= 2 after their producer where
the pair structure allows; remaining one-back consumers carry a serial
vchain wait.
"""

import numpy as np

B, C, H, W = 32, 256, 128, 128
N = H * W                 # 16384
NCORES = 8
BS = B // NCORES          # 4 batches per core
ROWS = BS * C             # 1024 rows per core
P = 128                   # partitions
NTILES = ROWS // P        # 8
SEG = 8192                # bf16 elems per segment (2MB per [128, SEG] tile)
NSEG = N // SEG           # 2
NSEGS = NTILES * NSEG     # 16
NSLOT = 9                 # SBUF ring depth
K = 8                     # top-K kept
RW = 10                   # rall cols per tile: 8 top + 1 sum + 1 pad
CH = 512                  # PE chunk width (one PSUM bank)
NCH = N // CH             # 32 chunks per tile
NBANK = 4                 # PSUM banks in rotation

_CACHE = {}


def _schedule():
    """Static DVE op order; 'ser'=1 marks one-back (serial) consumers."""
    ops = []
    ops += [("f1q0", 0, 0), ("f1q1", 0, 0), ("f1R0", 0, 0), ("f1", 1, 0),
            ("f2ab", 0, 1), ("f3ab", 0, 1), ("f4ab", 0, 1),
            ("m8", 0, 1), ("m8", 1, 0)]
    for k in (1, 2):
        a, b = 2 * k, 2 * k + 1
        ops += [("f1", a, 0), ("f1", b, 0), ("f2ab", a, 1), ("f3ab", a, 1),
                ("f4ab", a, 1), ("m8", a, 1), ("m8", b, 0)]
    # last pair: tiles 6/7 interleaved, tile 7 folds per DMA-half so only
    # the last half-segment's chain (~6.5us) remains after the final byte
    ops += [("f1", 6, 0), ("t7B", 7, 0), ("f2s", 6, 0), ("t7A", 7, 0),
            ("f3s", 6, 0), ("t7E", 7, 0), ("f4s", 6, 0), ("m8s", 6, 1),
            ("t7C", 7, 0), ("t7D", 7, 1), ("t7F", 7, 1), ("t7G", 7, 1),
            ("m8s", 7, 1)]
    ops += [("cast", -1, 1), ("mul", -1, 1), ("red", -1, 1)]
    pos = {}
    for i, op in enumerate(ops):
        pos[(op[0], op[1])] = i + 1
    return ops, pos


def _build():
    if "nc_raw" in _CACHE:
        return _CACHE["nc_raw"]
    from concourse import bacc, mybir

    f32 = mybir.dt.float32
    bf16 = mybir.dt.bfloat16
    Copy = mybir.ActivationFunctionType.Copy
    X = mybir.AxisListType.X
    add = mybir.AluOpType.add

    ops, pos = _schedule()
    V_TOTAL = len(ops)
    A_TOTAL = NTILES  # one PSUM-reduce ACT per tile

    # ring frees: seg j (< 8) must be released for seg j+8 by
    #  - DVE: its f1 read      - PE: its 16 chunk matmuls
    v_free = {}
    for t in range(4):          # tiles 0..3 own segs 0..7
        sL, sR = 2 * t, 2 * t + 1
        if t == 0:
            v_free[sL] = pos[("f1q1", 0)]
            v_free[sR] = pos[("f1R0", 0)]
        else:
            v_free[sL] = pos[("f1", t)]
            v_free[sR] = pos[("f1", t)]
    p_free = {j: 32 * (j // 2) + 16 * (j % 2 + 1) for j in range(NSEGS)}

    nc = bacc.Bacc(
        "TRN2", target_bir_lowering=False, debug=False, num_devices=NCORES
    )
    x = nc.dram_tensor("x", [ROWS, N], bf16, kind="ExternalInput").ap()
    cpk = nc.dram_tensor("cpk", [P, NTILES * RW], f32, kind="ExternalInput").ap()
    ident = nc.dram_tensor("ident", [P, P], bf16, kind="ExternalInput").ap()
    out = nc.dram_tensor("out", [P, NTILES], f32, kind="ExternalOutput").ap()
    import os
    dbg = bool(os.environ.get("KERNEL_DEBUG_RALL"))
    rall_out = (
        nc.dram_tensor("rall_out", [P, NTILES * RW], f32, kind="ExternalOutput").ap()
        if dbg
        else None
    )

    xbuf = nc.alloc_sbuf_tensor("xbuf", [P, NSLOT * SEG], bf16).ap()
    m1 = nc.alloc_sbuf_tensor("m1", [P, 2 * SEG], bf16).ap()   # [a | b]
    f2ab = nc.alloc_sbuf_tensor("f2ab", [P, SEG], bf16).ap()   # [2, 4096]
    f3ab = nc.alloc_sbuf_tensor("f3ab", [P, SEG // 2], bf16).ap()
    f4ab = nc.alloc_sbuf_tensor("f4ab", [P, SEG // 4], bf16).ap()
    idsb = nc.alloc_sbuf_tensor("idsb", [P, P], bf16).ap()
    dumact = nc.alloc_sbuf_tensor("dumact", [P, CH], bf16).ap()
    cand = nc.alloc_sbuf_tensor("cand", [P, NTILES * K], bf16).ap()
    rall = nc.alloc_sbuf_tensor("rall", [P, NTILES * RW], f32).ap()
    scr = nc.alloc_sbuf_tensor("scr", [P, NTILES * RW], f32).ap()
    cpksb = nc.alloc_sbuf_tensor("cpksb", [P, NTILES * RW], f32).ap()
    outsb = nc.alloc_sbuf_tensor("outsb", [P, NTILES], f32).ap()

    banks = [nc.alloc_psum_tensor(f"acc{i}", [P, CH], f32).ap() for i in range(NBANK)]

    seg_sem = [nc.alloc_semaphore(f"seg{k}") for k in range(NSLOT)]
    seg0a_sem = nc.alloc_semaphore("seg0a")
    s14a_sem = nc.alloc_semaphore("s14a")
    s15a_sem = nc.alloc_semaphore("s15a")
    cst_sem = nc.alloc_semaphore("cst")
    mset_sem = nc.alloc_semaphore("mset")
    out_sem = nc.alloc_semaphore("outd")
    vchain = nc.alloc_semaphore("vchain")
    achain = nc.alloc_semaphore("achain")
    pe_sem = nc.alloc_semaphore("pe_sem")

    def seg_thresh(i):
        return 16 * (i // NSLOT + 1)

    def slot(i):
        return xbuf[:, (i % NSLOT) * SEG : (i % NSLOT + 1) * SEG]

    with nc.Block(no_gpsimd_drain=True) as block:

        def issue_seg(eng, i):
            if i >= NSLOT:
                j = i - NSLOT
                if j in v_free:
                    eng.wait_ge(vchain, v_free[j])
                eng.wait_ge(pe_sem, p_free[j])
            t, sg = divmod(i, NSEG)
            src = x[t * P : (t + 1) * P, sg * SEG : (sg + 1) * SEG]
            if i in (0, 14, 15):
                half_sem = {0: seg0a_sem, 14: s14a_sem, 15: s15a_sem}[i]
                eng.dma_start(
                    out=slot(i)[:, 0 : SEG // 2], in_=src[:, 0 : SEG // 2]
                ).then_inc(half_sem, 16)
                eng.dma_start(
                    out=slot(i)[:, SEG // 2 : SEG], in_=src[:, SEG // 2 : SEG]
                ).then_inc(seg_sem[i % NSLOT], 16)
            else:
                eng.dma_start(out=slot(i), in_=src).then_inc(
                    seg_sem[i % NSLOT], 16
                )

        @block.sync
        def _(sync):
            for i in range(NSEGS):
                issue_seg(sync, i)
            sync.wait_ge(vchain, V_TOTAL)
            sync.dma_start(out=out[:], in_=outsb[:]).then_inc(out_sem, 16)
            if dbg:
                sync.dma_start(out=rall_out[:], in_=rall[:]).then_inc(out_sem, 16)
            sync.wait_ge(out_sem, 32 if dbg else 16)

        @block.gpsimd
        def _(g):
            g.memset(rall[:], 0.0).then_inc(mset_sem, 1)

        @block.tensor
        def _(te):
            te.wait_ge(cst_sem, 32)  # idsb loaded (scalar dma incs by 32)
            for t in range(NTILES):
                bank = banks[t % NBANK]
                if t >= NBANK:
                    te.wait_ge(achain, t - NBANK + 1)
                for c in range(NCH):
                    sg = 2 * t + c // 16
                    if c == 0:
                        if sg == 0:
                            te.wait_ge(seg0a_sem, 16)
                        elif sg == 14:
                            te.wait_ge(s14a_sem, 16)
                        else:
                            te.wait_ge(seg_sem[sg % NSLOT], seg_thresh(sg))
                    elif c == 8 and sg == 0:
                        te.wait_ge(seg_sem[0], 16)
                    elif c == 8 and sg == 14:
                        te.wait_ge(seg_sem[14 % NSLOT], seg_thresh(14))
                    elif c == 16:
                        if sg == 15:
                            te.wait_ge(s15a_sem, 16)
                        else:
                            te.wait_ge(seg_sem[sg % NSLOT], seg_thresh(sg))
                    elif c == 24 and sg == 15:
                        te.wait_ge(seg_sem[15 % NSLOT], seg_thresh(15))
                    off = (c % 16) * CH
                    te.matmul(
                        bank[:],
                        idsb[:],
                        slot(sg)[:, off : off + CH],
                        start=(c == 0),
                        stop=(c == NCH - 1),
                    ).then_inc(pe_sem, 1)

        @block.scalar
        def _(s):
            s.dma_start(out=cpksb[:], in_=cpk[:]).then_inc(cst_sem, 16)
            s.dma_start(out=idsb[:], in_=ident[:]).then_inc(cst_sem, 16)
            s.wait_ge(mset_sem, 1)
            for t in range(NTILES):
                s.wait_ge(pe_sem, 32 * (t + 1))
                ins = s.activation(
                    dumact[:],
                    banks[t % NBANK][:],
                    Copy,
                    bias=0.0,
                    scale=1.0,
                    accum_out=rall[:, t * RW + K : t * RW + K + 1],
                )
                if t >= 2:
                    ins._wait_ge(achain, t - 1)
                ins.then_inc(achain)

        @block.vector
        def _(v):
            vc = 0

            def emit(ins, serial=False):
                nonlocal vc
                ins._wait_ge(vchain, vc if serial else max(0, vc - 1))
                ins.then_inc(vchain)
                vc += 1

            v.wait_ge(cst_sem, 32)
            v.wait_ge(mset_sem, 1)

            for kind, t, ser in ops:
                h = t % 2
                mt = m1[:, h * SEG : (h + 1) * SEG]
                if kind == "f1q0":
                    v.wait_ge(seg0a_sem, 16)
                    emit(v.tensor_max(
                        m1[:, 0 : SEG // 4],
                        xbuf[:, 0 : SEG // 4],
                        xbuf[:, SEG // 4 : SEG // 2],
                    ))
                elif kind == "f1q1":
                    v.wait_ge(seg_sem[0], 16)
                    emit(v.tensor_max(
                        m1[:, SEG // 4 : SEG // 2],
                        xbuf[:, SEG // 2 : SEG // 2 + SEG // 4],
                        xbuf[:, SEG // 2 + SEG // 4 : SEG],
                    ))
                elif kind == "f1R0":
                    v.wait_ge(seg_sem[1], 16)
                    emit(v.tensor_max(
                        m1[:, SEG // 2 : SEG],
                        xbuf[:, SEG : SEG + SEG // 2],
                        xbuf[:, SEG + SEG // 2 : 2 * SEG],
                    ))
                elif kind == "f1":
                    sL, sR = 2 * t, 2 * t + 1
                    v.wait_ge(seg_sem[sL % NSLOT], seg_thresh(sL))
                    v.wait_ge(seg_sem[sR % NSLOT], seg_thresh(sR))
                    emit(v.tensor_max(mt, slot(sL), slot(sR)))
                elif kind == "t7A":
                    # fold within seg14 -> m1b[0:4096]
                    v.wait_ge(s14a_sem, 16)
                    v.wait_ge(seg_sem[14 % NSLOT], seg_thresh(14))
                    emit(v.tensor_max(
                        mt[:, 0 : SEG // 2],
                        slot(14)[:, 0 : SEG // 2],
                        slot(14)[:, SEG // 2 : SEG],
                    ))
                elif kind == "t7B":
                    # fold within seg15 first half -> m1b[4096:6144]
                    v.wait_ge(s15a_sem, 16)
                    emit(v.tensor_max(
                        mt[:, SEG // 2 : SEG // 2 + SEG // 4],
                        slot(15)[:, 0 : SEG // 4],
                        slot(15)[:, SEG // 4 : SEG // 2],
                    ))
                elif kind == "t7C":
                    # fold within seg15 second half -> m1b[6144:8192]
                    v.wait_ge(seg_sem[15 % NSLOT], seg_thresh(15))
                    emit(v.tensor_max(
                        mt[:, SEG // 2 + SEG // 4 : SEG],
                        slot(15)[:, SEG // 2 : SEG // 2 + SEG // 4],
                        slot(15)[:, SEG // 2 + SEG // 4 : SEG],
                    ))
                elif kind == "t7E":
                    # E = max(fold14.lo, fold14.hi) -> f2ab[4096:6144]
                    emit(v.tensor_max(
                        f2ab[:, SEG // 2 : SEG // 2 + SEG // 4],
                        mt[:, 0 : SEG // 4],
                        mt[:, SEG // 4 : SEG // 2],
                    ), serial=bool(ser))
                elif kind == "t7D":
                    # D = max(fold15lo, fold15hi) -> f2ab[6144:8192]
                    emit(v.tensor_max(
                        f2ab[:, SEG // 2 + SEG // 4 : SEG],
                        mt[:, SEG // 2 : SEG // 2 + SEG // 4],
                        mt[:, SEG // 2 + SEG // 4 : SEG],
                    ), serial=bool(ser))
                elif kind == "t7F":
                    # F = max(E, D) -> f3ab[2048:4096]
                    emit(v.tensor_max(
                        f3ab[:, SEG // 4 : SEG // 2],
                        f2ab[:, SEG // 2 : SEG // 2 + SEG // 4],
                        f2ab[:, SEG // 2 + SEG // 4 : SEG],
                    ), serial=bool(ser))
                elif kind == "t7G":
                    # G = fold(F) -> f4ab[1024:2048]
                    emit(v.tensor_max(
                        f4ab[:, SEG // 8 : SEG // 4],
                        f3ab[:, SEG // 4 : SEG // 4 + SEG // 8],
                        f3ab[:, SEG // 4 + SEG // 8 : SEG // 2],
                    ), serial=bool(ser))
                elif kind == "f2ab":
                    emit(v.tensor_max(
                        f2ab.rearrange("p (g w) -> p g w", w=SEG // 2),
                        m1.rearrange("p (g w) -> p g w", w=SEG)[:, :, 0 : SEG // 2],
                        m1.rearrange("p (g w) -> p g w", w=SEG)[:, :, SEG // 2 : SEG],
                    ), serial=bool(ser))
                elif kind == "f3ab":
                    emit(v.tensor_max(
                        f3ab.rearrange("p (g w) -> p g w", w=SEG // 4),
                        f2ab.rearrange("p (g w) -> p g w", w=SEG // 2)[:, :, 0 : SEG // 4],
                        f2ab.rearrange("p (g w) -> p g w", w=SEG // 2)[:, :, SEG // 4 : SEG // 2],
                    ), serial=bool(ser))
                elif kind == "f4ab":
                    emit(v.tensor_max(
                        f4ab.rearrange("p (g w) -> p g w", w=SEG // 8),
                        f3ab.rearrange("p (g w) -> p g w", w=SEG // 4)[:, :, 0 : SEG // 8],
                        f3ab.rearrange("p (g w) -> p g w", w=SEG // 4)[:, :, SEG // 8 : SEG // 4],
                    ), serial=bool(ser))
                elif kind == "m8":
                    emit(v.max(
                        cand[:, t * K : (t + 1) * K],
                        f4ab[:, h * (SEG // 8) : (h + 1) * (SEG // 8)],
                    ), serial=bool(ser))
                elif kind == "f2s":
                    emit(v.tensor_max(
                        f2ab[:, h * (SEG // 2) : (h + 1) * (SEG // 2)],
                        mt[:, 0 : SEG // 2], mt[:, SEG // 2 : SEG],
                    ), serial=bool(ser))
                elif kind == "f3s":
                    fs = f2ab[:, h * (SEG // 2) : (h + 1) * (SEG // 2)]
                    emit(v.tensor_max(
                        f3ab[:, h * (SEG // 4) : (h + 1) * (SEG // 4)],
                        fs[:, 0 : SEG // 4], fs[:, SEG // 4 : SEG // 2],
                    ), serial=bool(ser))
                elif kind == "f4s":
                    fs = f3ab[:, h * (SEG // 4) : (h + 1) * (SEG // 4)]
                    emit(v.tensor_max(
                        f4ab[:, h * (SEG // 8) : (h + 1) * (SEG // 8)],
                        fs[:, 0 : SEG // 8], fs[:, SEG // 8 : SEG // 4],
                    ), serial=bool(ser))
                elif kind == "m8s":
                    emit(v.max(
                        cand[:, t * K : (t + 1) * K],
                        f4ab[:, h * (SEG // 8) : (h + 1) * (SEG // 8)],
                    ), serial=bool(ser))
                elif kind == "cast":
                    emit(v.tensor_copy(
                        rall.rearrange("p (t r) -> p t r", r=RW)[:, :, 0:K],
                        cand.rearrange("p (t k) -> p t k", k=K),
                    ), serial=True)
                elif kind == "mul":
                    v.wait_ge(achain, A_TOTAL)
                    emit(v.tensor_mul(scr[:], rall[:], cpksb[:]), serial=True)
                elif kind == "red":
                    emit(v.tensor_reduce(
                        outsb[:],
                        scr.rearrange("p (t r) -> p t r", r=RW),
                        axis=X,
                        op=add,
                    ), serial=True)
            assert vc == V_TOTAL, (vc, V_TOTAL)

    nc.compile()
    _CACHE["nc_raw"] = nc
    return nc


def _host_weights(dc_logit: np.ndarray):
    """wu2[c, 0:8] = (w[c,j]-0.5)/sum_w[c]; col 8 = 0.5/sum_w; col 9 = 0."""
    dc = dc_logit.astype(np.float64)
    j = np.arange(N, dtype=np.float64)
    pw = dc[:, None] ** j[None, :]
    wfull = (1.0 / (1.0 + np.exp(-pw))).astype(np.float32)  # [C, N]
    dev = np.abs(wfull[:, K:] - np.float32(0.5))
    assert dev.max() < 2e-4, f"top-{K} decomposition invalid: {dev.max()}"
    sum_w = wfull.astype(np.float64).sum(axis=1)
    winv = 1.0 / sum_w
    wu2 = np.zeros((C, RW), np.float32)
    wu2[:, :K] = ((wfull[:, :K].astype(np.float64) - 0.5) * winv[:, None]).astype(
        np.float32
    )
    wu2[:, K] = (0.5 * winv).astype(np.float32)
    return wu2


def _run_pjrt(nc, in_maps):
    """Pre-uploads all inputs to the devices before dispatching the NEFF."""
    import jax
    import numpy as np
    from jax.sharding import Mesh, NamedSharding, PartitionSpec
    from jax.experimental.shard_map import shard_map
    from concourse import bass2jax, mybir

    bass2jax.install_neuronx_cc_hook()
    assert nc.dbg_addr is None
    n_cores = len(in_maps)
    partition_name = (
        nc.partition_id_tensor.name if nc.partition_id_tensor else None
    )

    in_names, out_names, out_avals, zero_outs = [], [], [], []
    for alloc in nc.m.functions[0].allocations:
        if not isinstance(alloc, mybir.MemoryLocationSet):
            continue
        name = alloc.memorylocations[0].name
        if alloc.kind == "ExternalInput":
            if name != partition_name:
                in_names.append(name)
        elif alloc.kind == "ExternalOutput":
            shape = tuple(alloc.tensor_shape)
            dtype = mybir.dt.np(alloc.dtype)
            out_names.append(name)
            out_avals.append(jax.core.ShapedArray(shape, dtype))
            zero_outs.append(np.zeros(shape, dtype))
    n_params = len(in_names)
    n_outs = len(out_avals)
    all_in_names = list(in_names) + out_names
    if partition_name is not None:
        all_in_names.append(partition_name)
    donate = tuple(range(n_params, n_params + n_outs))

    def _body(*args):
        operands = list(args)
        if partition_name is not None:
            operands.append(bass2jax.partition_id_tensor())
        return tuple(
            bass2jax._bass_exec_p.bind(
                *operands,
                out_avals=tuple(out_avals),
                in_names=tuple(all_in_names),
                out_names=tuple(out_names),
                lowering_input_output_aliases=(),
                sim_require_finite=True,
                sim_require_nnan=True,
                nc=nc,
            )
        )

    devices = jax.devices()[:n_cores]
    mesh = Mesh(np.asarray(devices), ("core",))
    spec = PartitionSpec("core")
    sharded = jax.jit(
        shard_map(
            _body,
            mesh=mesh,
            in_specs=(spec,) * (n_params + n_outs),
            out_specs=(spec,) * n_outs,
            check_rep=False,
        ),
        donate_argnums=donate,
        keep_unused=True,
    )
    sh = NamedSharding(mesh, spec)
    concat_in = [
        jax.device_put(
            np.concatenate([np.asarray(in_maps[c][k]) for c in range(n_cores)], axis=0),
            sh,
        )
        for k in in_names
    ]
    concat_zeros = [
        jax.device_put(
            np.zeros((n_cores * z.shape[0], *z.shape[1:]), z.dtype), sh
        )
        for z in zero_outs
    ]
    jax.block_until_ready(concat_in)
    jax.block_until_ready(concat_zeros)
    out_arrs = sharded(*concat_in, *concat_zeros)
    return [
        {
            name: np.asarray(out_arrs[i]).reshape(n_cores, *out_avals[i].shape)[c]
            for i, name in enumerate(out_names)
        }
        for c in range(n_cores)
    ]


def _in_maps(x: np.ndarray, dc_logit: np.ndarray):
    import ml_dtypes

    wu2 = _host_weights(np.asarray(dc_logit))  # [C, RW]
    cpk = np.empty((P, NTILES * RW), np.float32)
    for t in range(NTILES):
        cpk[:, t * RW : (t + 1) * RW] = wu2[(t % 2) * P : (t % 2 + 1) * P]
    ident = np.eye(P, dtype=np.float32).astype(ml_dtypes.bfloat16)
    xr = np.ascontiguousarray(x).reshape(B * C, N).astype(ml_dtypes.bfloat16)
    return [
        {"x": xr[i * ROWS : (i + 1) * ROWS], "cpk": cpk, "ident": ident}
        for i in range(NCORES)
    ]


def kernel(x: np.ndarray, dc_logit: np.ndarray) -> np.ndarray:
    import time

    nc = _build()
    in_maps = _in_maps(x, dc_logit)
    last_err = None
    for attempt in range(3):
        try:
            results = _run_pjrt(nc, in_maps)
            break
        except Exception as e:  # transient device errors (wedged core etc.)
            last_err = e
            time.sleep(15)
    else:
        raise last_err
    outs = []
    for i in range(NCORES):
        o = results[i]["out"]  # [P, NTILES]; col t, row p -> global row t*128+p
        outs.append(o.T.reshape(BS, C))
    return np.concatenate(outs, axis=0).astype(np.float32)


# revision 20
# speedup vs baseline: 1.1423x; 1.0188x over previous
"""AdaptiveGlobalWeightedRankPooling2d on 8 Trainium2 NeuronCores.

Math: y[b,c] = sum_n sort_desc(x[b,c])[n] * w[c,n] / sum_n w[c,n]
with w[c,n] = sigmoid(dc_logit[c] ** n).  In f32, w[c,n] == 0.5 exactly
for n >= 18 (dc_logit ~ 0.4055); ranks >= 8 deviate by < 2e-4 in total
weight, so

    y[b,c] = sum_{j<8} top_j * wu[c,j]  +  S[b,c] * (0.5 / sum_w[c])

with wu[c,j] = (w[c,j]-0.5)/sum_w[c] host-precomputed and S the full row
sum.  x is staged to the device as bf16 (rel-err floor ~1.7e-3, gate
2e-2; host-validated pipeline ~2.0e-3).

Per core: 1024 rows of N=16384 bf16 = 8 tiles x [128, 16384], DMAed as
16 2MB segments into an 8-slot SBUF ring.

Engine split (measured op costs):
  PE (idle otherwise) computes ALL row sums: identity-stationary
    matmuls accumulate 32 x [128, 512] chunks per tile into a PSUM bank
    (A[p,f] += x[p, 512c+f]; 216ns/chunk steady state, ~55us total).
    Numerically exact f32 accumulation of the bf16 values.
  ScalarE reduces each tile's [128, 512] PSUM bank with one
    activation-accumulate into rall (~0.9us/tile) - else idle.
  DVE owns the max path: per tile pair, f1_a/f1_b (8192-out 2x
    tensor_max), merged f2ab/f3ab/f4ab folds -> [2, 1024], then max8
    (sorted top-8) per tile; end: cast top8s->f32, 80-wide mul by host
    weights, grouped reduce -> out [128, 8].  ~80us busy = critical
    engine.  The last pair is interleaved per-tile with seg14/15 DMAed
    in halves so only ~6.5us of DVE work trails the final byte.

Measured: typical core ~109us (vs 125.4us baseline); occasional cores
+15us from cross-core HBM/DMA arbitration jitter (run-to-run, not
deterministic) - test.py reports best-of-3 repetitions of the worst
core to suppress that noise.  rel err 1.695e-3 (pure bf16 floor; sums
are exact f32 via PE).

DVE write-ack pipelining: ops are spaced >= 2 after their producer where
the pair structure allows; remaining one-back consumers carry a serial
vchain wait.
"""

import numpy as np

B, C, H, W = 32, 256, 128, 128
N = H * W                 # 16384
NCORES = 8
BS = B // NCORES          # 4 batches per core
ROWS = BS * C             # 1024 rows per core
P = 128                   # partitions
NTILES = ROWS // P        # 8
SEG = 8192                # bf16 elems per segment (2MB per [128, SEG] tile)
NSEG = N // SEG           # 2
NSEGS = NTILES * NSEG     # 16
NSLOT = 9                 # SBUF ring depth
K = 8                     # top-K kept
RW = 10                   # rall cols per tile: 8 top + 1 sum + 1 pad
CH = 512                  # PE chunk width (one PSUM bank)
NCH = N // CH             # 32 chunks per tile
NBANK = 4                 # PSUM banks in rotation

_CACHE = {}


def _schedule():
    """Static DVE op order; 'ser'=1 marks one-back (serial) consumers."""
    ops = []
    ops += [("f1q0", 0, 0), ("f1q1", 0, 0), ("f1R0", 0, 0), ("f1", 1, 0),
            ("f2ab", 0, 1), ("f3ab", 0, 1), ("f4ab", 0, 1),
            ("m8", 0, 1), ("m8", 1, 0)]
    for k in (1, 2):
        a, b = 2 * k, 2 * k + 1
        ops += [("f1", a, 0), ("f1", b, 0), ("f2ab", a, 1), ("f3ab", a, 1),
                ("f4ab", a, 1), ("m8", a, 1), ("m8", b, 0)]
    # last pair: tiles 6/7 interleaved, tile 7 folds per DMA-half so only
    # the last half-segment's chain (~6.5us) remains after the final byte
    ops += [("f1", 6, 0), ("t7B", 7, 0), ("f2s", 6, 0), ("t7A", 7, 0),
            ("f3s", 6, 0), ("t7E", 7, 0), ("f4s", 6, 0), ("m8s", 6, 1),
            ("t7C", 7, 0), ("t7D", 7, 1), ("t7F", 7, 1), ("t7G", 7, 1),
            ("m8s", 7, 1)]
    ops += [("cast", -1, 1), ("mul", -1, 1), ("red", -1, 1)]
    pos = {}
    for i, op in enumerate(ops):
        pos[(op[0], op[1])] = i + 1
    return ops, pos


def _build():
    if "nc_raw" in _CACHE:
        return _CACHE["nc_raw"]
    from concourse import bacc, mybir

    f32 = mybir.dt.float32
    bf16 = mybir.dt.bfloat16
    Copy = mybir.ActivationFunctionType.Copy
    X = mybir.AxisListType.X
    add = mybir.AluOpType.add

    ops, pos = _schedule()
    V_TOTAL = len(ops)
    A_TOTAL = NTILES  # one PSUM-reduce ACT per tile

    # ring frees: seg j (< 8) must be released for seg j+8 by
    #  - DVE: its f1 read      - PE: its 16 chunk matmuls
    v_free = {}
    for t in range(4):          # tiles 0..3 own segs 0..7
        sL, sR = 2 * t, 2 * t + 1
        if t == 0:
            v_free[sL] = pos[("f1q1", 0)]
            v_free[sR] = pos[("f1R0", 0)]
        else:
            v_free[sL] = pos[("f1", t)]
            v_free[sR] = pos[("f1", t)]
    p_free = {j: 32 * (j // 2) + 16 * (j % 2 + 1) for j in range(NSEGS)}

    nc = bacc.Bacc(
        "TRN2", target_bir_lowering=False, debug=False, num_devices=NCORES
    )
    x = nc.dram_tensor("x", [ROWS, N], bf16, kind="ExternalInput").ap()
    cpk = nc.dram_tensor("cpk", [P, NTILES * RW], f32, kind="ExternalInput").ap()
    ident = nc.dram_tensor("ident", [P, P], bf16, kind="ExternalInput").ap()
    out = nc.dram_tensor("out", [P, NTILES], f32, kind="ExternalOutput").ap()
    import os
    dbg = bool(os.environ.get("KERNEL_DEBUG_RALL"))
    rall_out = (
        nc.dram_tensor("rall_out", [P, NTILES * RW], f32, kind="ExternalOutput").ap()
        if dbg
        else None
    )

    xbuf = nc.alloc_sbuf_tensor("xbuf", [P, NSLOT * SEG], bf16).ap()
    m1 = nc.alloc_sbuf_tensor("m1", [P, 2 * SEG], bf16).ap()   # [a | b]
    f2ab = nc.alloc_sbuf_tensor("f2ab", [P, SEG], bf16).ap()   # [2, 4096]
    f3ab = nc.alloc_sbuf_tensor("f3ab", [P, SEG // 2], bf16).ap()
    f4ab = nc.alloc_sbuf_tensor("f4ab", [P, SEG // 4], bf16).ap()
    idsb = nc.alloc_sbuf_tensor("idsb", [P, P], bf16).ap()
    dumact = nc.alloc_sbuf_tensor("dumact", [P, CH], bf16).ap()
    cand = nc.alloc_sbuf_tensor("cand", [P, NTILES * K], bf16).ap()
    rall = nc.alloc_sbuf_tensor("rall", [P, NTILES * RW], f32).ap()
    scr = nc.alloc_sbuf_tensor("scr", [P, NTILES * RW], f32).ap()
    cpksb = nc.alloc_sbuf_tensor("cpksb", [P, NTILES * RW], f32).ap()
    outsb = nc.alloc_sbuf_tensor("outsb", [P, NTILES], f32).ap()

    banks = [nc.alloc_psum_tensor(f"acc{i}", [P, CH], f32).ap() for i in range(NBANK)]

    seg_sem = [nc.alloc_semaphore(f"seg{k}") for k in range(NSLOT)]
    cst_sem = nc.alloc_semaphore("cst")
    mset_sem = nc.alloc_semaphore("mset")
    out_sem = nc.alloc_semaphore("outd")
    vchain = nc.alloc_semaphore("vchain")
    achain = nc.alloc_semaphore("achain")
    pe_sem = nc.alloc_semaphore("pe_sem")

    # every segment is DMAed as two 1MB halves, each +16 on its slot sem:
    # full seg present at base(i)+32, first half at base(i)+16
    def seg_base(i):
        return 32 * (i // NSLOT)

    def seg_thresh(i):
        return seg_base(i) + 32

    def slot(i):
        return xbuf[:, (i % NSLOT) * SEG : (i % NSLOT + 1) * SEG]

    with nc.Block(no_gpsimd_drain=True) as block:

        def issue_seg(eng, i):
            if i >= NSLOT:
                j = i - NSLOT
                if j in v_free:
                    eng.wait_ge(vchain, v_free[j])
                eng.wait_ge(pe_sem, p_free[j])
            t, sg = divmod(i, NSEG)
            src = x[t * P : (t + 1) * P, sg * SEG : (sg + 1) * SEG]
            eng.dma_start(
                out=slot(i)[:, 0 : SEG // 2], in_=src[:, 0 : SEG // 2]
            ).then_inc(seg_sem[i % NSLOT], 16)
            eng.dma_start(
                out=slot(i)[:, SEG // 2 : SEG], in_=src[:, SEG // 2 : SEG]
            ).then_inc(seg_sem[i % NSLOT], 16)

        @block.sync
        def _(sync):
            for i in range(NSEGS):
                issue_seg(sync, i)
            sync.wait_ge(vchain, V_TOTAL)
            sync.dma_start(out=out[:], in_=outsb[:]).then_inc(out_sem, 16)
            if dbg:
                sync.dma_start(out=rall_out[:], in_=rall[:]).then_inc(out_sem, 16)
            sync.wait_ge(out_sem, 32 if dbg else 16)

        @block.gpsimd
        def _(g):
            g.memset(rall[:], 0.0).then_inc(mset_sem, 1)

        @block.tensor
        def _(te):
            te.wait_ge(cst_sem, 32)  # idsb loaded (scalar dma incs by 32)
            for t in range(NTILES):
                bank = banks[t % NBANK]
                if t >= NBANK:
                    te.wait_ge(achain, t - NBANK + 1)
                for c in range(NCH):
                    sg = 2 * t + c // 16
                    if c % 8 == 0:
                        # chunks 0-7 use the seg's first 1MB half, 8-15 the rest
                        half = (c % 16) // 8
                        te.wait_ge(
                            seg_sem[sg % NSLOT], seg_base(sg) + 16 * (half + 1)
                        )
                    off = (c % 16) * CH
                    te.matmul(
                        bank[:],
                        idsb[:],
                        slot(sg)[:, off : off + CH],
                        start=(c == 0),
                        stop=(c == NCH - 1),
                    ).then_inc(pe_sem, 1)

        @block.scalar
        def _(s):
            s.dma_start(out=cpksb[:], in_=cpk[:]).then_inc(cst_sem, 16)
            s.dma_start(out=idsb[:], in_=ident[:]).then_inc(cst_sem, 16)
            s.wait_ge(mset_sem, 1)
            for t in range(NTILES):
                s.wait_ge(pe_sem, 32 * (t + 1))
                ins = s.activation(
                    dumact[:],
                    banks[t % NBANK][:],
                    Copy,
                    bias=0.0,
                    scale=1.0,
                    accum_out=rall[:, t * RW + K : t * RW + K + 1],
                )
                if t >= 2:
                    ins._wait_ge(achain, t - 1)
                ins.then_inc(achain)

        @block.vector
        def _(v):
            vc = 0

            def emit(ins, serial=False):
                nonlocal vc
                ins._wait_ge(vchain, vc if serial else max(0, vc - 1))
                ins.then_inc(vchain)
                vc += 1

            v.wait_ge(cst_sem, 32)
            v.wait_ge(mset_sem, 1)

            for kind, t, ser in ops:
                h = t % 2
                mt = m1[:, h * SEG : (h + 1) * SEG]
                if kind == "f1q0":
                    v.wait_ge(seg_sem[0], 16)
                    emit(v.tensor_max(
                        m1[:, 0 : SEG // 4],
                        xbuf[:, 0 : SEG // 4],
                        xbuf[:, SEG // 4 : SEG // 2],
                    ))
                elif kind == "f1q1":
                    v.wait_ge(seg_sem[0], 32)
                    emit(v.tensor_max(
                        m1[:, SEG // 4 : SEG // 2],
                        xbuf[:, SEG // 2 : SEG // 2 + SEG // 4],
                        xbuf[:, SEG // 2 + SEG // 4 : SEG],
                    ))
                elif kind == "f1R0":
                    v.wait_ge(seg_sem[1], 32)
                    emit(v.tensor_max(
                        m1[:, SEG // 2 : SEG],
                        xbuf[:, SEG : SEG + SEG // 2],
                        xbuf[:, SEG + SEG // 2 : 2 * SEG],
                    ))
                elif kind == "f1":
                    sL, sR = 2 * t, 2 * t + 1
                    v.wait_ge(seg_sem[sL % NSLOT], seg_thresh(sL))
                    v.wait_ge(seg_sem[sR % NSLOT], seg_thresh(sR))
                    emit(v.tensor_max(mt, slot(sL), slot(sR)))
                elif kind == "t7A":
                    # fold within seg14 -> m1b[0:4096]
                    v.wait_ge(seg_sem[14 % NSLOT], seg_thresh(14))
                    emit(v.tensor_max(
                        mt[:, 0 : SEG // 2],
                        slot(14)[:, 0 : SEG // 2],
                        slot(14)[:, SEG // 2 : SEG],
                    ))
                elif kind == "t7B":
                    # fold within seg15 first half -> m1b[4096:6144]
                    v.wait_ge(seg_sem[15 % NSLOT], seg_base(15) + 16)
                    emit(v.tensor_max(
                        mt[:, SEG // 2 : SEG // 2 + SEG // 4],
                        slot(15)[:, 0 : SEG // 4],
                        slot(15)[:, SEG // 4 : SEG // 2],
                    ))
                elif kind == "t7C":
                    # fold within seg15 second half -> m1b[6144:8192]
                    v.wait_ge(seg_sem[15 % NSLOT], seg_thresh(15))
                    emit(v.tensor_max(
                        mt[:, SEG // 2 + SEG // 4 : SEG],
                        slot(15)[:, SEG // 2 : SEG // 2 + SEG // 4],
                        slot(15)[:, SEG // 2 + SEG // 4 : SEG],
                    ))
                elif kind == "t7E":
                    # E = max(fold14.lo, fold14.hi) -> f2ab[4096:6144]
                    emit(v.tensor_max(
                        f2ab[:, SEG // 2 : SEG // 2 + SEG // 4],
                        mt[:, 0 : SEG // 4],
                        mt[:, SEG // 4 : SEG // 2],
                    ), serial=bool(ser))
                elif kind == "t7D":
                    # D = max(fold15lo, fold15hi) -> f2ab[6144:8192]
                    emit(v.tensor_max(
                        f2ab[:, SEG // 2 + SEG // 4 : SEG],
                        mt[:, SEG // 2 : SEG // 2 + SEG // 4],
                        mt[:, SEG // 2 + SEG // 4 : SEG],
                    ), serial=bool(ser))
                elif kind == "t7F":
                    # F = max(E, D) -> f3ab[2048:4096]
                    emit(v.tensor_max(
                        f3ab[:, SEG // 4 : SEG // 2],
                        f2ab[:, SEG // 2 : SEG // 2 + SEG // 4],
                        f2ab[:, SEG // 2 + SEG // 4 : SEG],
                    ), serial=bool(ser))
                elif kind == "t7G":
                    # G = fold(F) -> f4ab[1024:2048]
                    emit(v.tensor_max(
                        f4ab[:, SEG // 8 : SEG // 4],
                        f3ab[:, SEG // 4 : SEG // 4 + SEG // 8],
                        f3ab[:, SEG // 4 + SEG // 8 : SEG // 2],
                    ), serial=bool(ser))
                elif kind == "f2ab":
                    emit(v.tensor_max(
                        f2ab.rearrange("p (g w) -> p g w", w=SEG // 2),
                        m1.rearrange("p (g w) -> p g w", w=SEG)[:, :, 0 : SEG // 2],
                        m1.rearrange("p (g w) -> p g w", w=SEG)[:, :, SEG // 2 : SEG],
                    ), serial=bool(ser))
                elif kind == "f3ab":
                    emit(v.tensor_max(
                        f3ab.rearrange("p (g w) -> p g w", w=SEG // 4),
                        f2ab.rearrange("p (g w) -> p g w", w=SEG // 2)[:, :, 0 : SEG // 4],
                        f2ab.rearrange("p (g w) -> p g w", w=SEG // 2)[:, :, SEG // 4 : SEG // 2],
                    ), serial=bool(ser))
                elif kind == "f4ab":
                    emit(v.tensor_max(
                        f4ab.rearrange("p (g w) -> p g w", w=SEG // 8),
                        f3ab.rearrange("p (g w) -> p g w", w=SEG // 4)[:, :, 0 : SEG // 8],
                        f3ab.rearrange("p (g w) -> p g w", w=SEG // 4)[:, :, SEG // 8 : SEG // 4],
                    ), serial=bool(ser))
                elif kind == "m8":
                    emit(v.max(
                        cand[:, t * K : (t + 1) * K],
                        f4ab[:, h * (SEG // 8) : (h + 1) * (SEG // 8)],
                    ), serial=bool(ser))
                elif kind == "f2s":
                    emit(v.tensor_max(
                        f2ab[:, h * (SEG // 2) : (h + 1) * (SEG // 2)],
                        mt[:, 0 : SEG // 2], mt[:, SEG // 2 : SEG],
                    ), serial=bool(ser))
                elif kind == "f3s":
                    fs = f2ab[:, h * (SEG // 2) : (h + 1) * (SEG // 2)]
                    emit(v.tensor_max(
                        f3ab[:, h * (SEG // 4) : (h + 1) * (SEG // 4)],
                        fs[:, 0 : SEG // 4], fs[:, SEG // 4 : SEG // 2],
                    ), serial=bool(ser))
                elif kind == "f4s":
                    fs = f3ab[:, h * (SEG // 4) : (h + 1) * (SEG // 4)]
                    emit(v.tensor_max(
                        f4ab[:, h * (SEG // 8) : (h + 1) * (SEG // 8)],
                        fs[:, 0 : SEG // 8], fs[:, SEG // 8 : SEG // 4],
                    ), serial=bool(ser))
                elif kind == "m8s":
                    emit(v.max(
                        cand[:, t * K : (t + 1) * K],
                        f4ab[:, h * (SEG // 8) : (h + 1) * (SEG // 8)],
                    ), serial=bool(ser))
                elif kind == "cast":
                    emit(v.tensor_copy(
                        rall.rearrange("p (t r) -> p t r", r=RW)[:, :, 0:K],
                        cand.rearrange("p (t k) -> p t k", k=K),
                    ), serial=True)
                elif kind == "mul":
                    v.wait_ge(achain, A_TOTAL)
                    emit(v.tensor_mul(scr[:], rall[:], cpksb[:]), serial=True)
                elif kind == "red":
                    emit(v.tensor_reduce(
                        outsb[:],
                        scr.rearrange("p (t r) -> p t r", r=RW),
                        axis=X,
                        op=add,
                    ), serial=True)
            assert vc == V_TOTAL, (vc, V_TOTAL)

    nc.compile()
    _CACHE["nc_raw"] = nc
    return nc


def _host_weights(dc_logit: np.ndarray):
    """wu2[c, 0:8] = (w[c,j]-0.5)/sum_w[c]; col 8 = 0.5/sum_w; col 9 = 0."""
    dc = dc_logit.astype(np.float64)
    j = np.arange(N, dtype=np.float64)
    pw = dc[:, None] ** j[None, :]
    wfull = (1.0 / (1.0 + np.exp(-pw))).astype(np.float32)  # [C, N]
    dev = np.abs(wfull[:, K:] - np.float32(0.5))
    assert dev.max() < 2e-4, f"top-{K} decomposition invalid: {dev.max()}"
    sum_w = wfull.astype(np.float64).sum(axis=1)
    winv = 1.0 / sum_w
    wu2 = np.zeros((C, RW), np.float32)
    wu2[:, :K] = ((wfull[:, :K].astype(np.float64) - 0.5) * winv[:, None]).astype(
        np.float32
    )
    wu2[:, K] = (0.5 * winv).astype(np.float32)
    return wu2


def _run_pjrt(nc, in_maps):
    """Pre-uploads all inputs to the devices before dispatching the NEFF."""
    import jax
    import numpy as np
    from jax.sharding import Mesh, NamedSharding, PartitionSpec
    from jax.experimental.shard_map import shard_map
    from concourse import bass2jax, mybir

    bass2jax.install_neuronx_cc_hook()
    assert nc.dbg_addr is None
    n_cores = len(in_maps)
    partition_name = (
        nc.partition_id_tensor.name if nc.partition_id_tensor else None
    )

    in_names, out_names, out_avals, zero_outs = [], [], [], []
    for alloc in nc.m.functions[0].allocations:
        if not isinstance(alloc, mybir.MemoryLocationSet):
            continue
        name = alloc.memorylocations[0].name
        if alloc.kind == "ExternalInput":
            if name != partition_name:
                in_names.append(name)
        elif alloc.kind == "ExternalOutput":
            shape = tuple(alloc.tensor_shape)
            dtype = mybir.dt.np(alloc.dtype)
            out_names.append(name)
            out_avals.append(jax.core.ShapedArray(shape, dtype))
            zero_outs.append(np.zeros(shape, dtype))
    n_params = len(in_names)
    n_outs = len(out_avals)
    all_in_names = list(in_names) + out_names
    if partition_name is not None:
        all_in_names.append(partition_name)
    donate = tuple(range(n_params, n_params + n_outs))

    def _body(*args):
        operands = list(args)
        if partition_name is not None:
            operands.append(bass2jax.partition_id_tensor())
        return tuple(
            bass2jax._bass_exec_p.bind(
                *operands,
                out_avals=tuple(out_avals),
                in_names=tuple(all_in_names),
                out_names=tuple(out_names),
                lowering_input_output_aliases=(),
                sim_require_finite=True,
                sim_require_nnan=True,
                nc=nc,
            )
        )

    devices = jax.devices()[:n_cores]
    mesh = Mesh(np.asarray(devices), ("core",))
    spec = PartitionSpec("core")
    sharded = jax.jit(
        shard_map(
            _body,
            mesh=mesh,
            in_specs=(spec,) * (n_params + n_outs),
            out_specs=(spec,) * n_outs,
            check_rep=False,
        ),
        donate_argnums=donate,
        keep_unused=True,
    )
    sh = NamedSharding(mesh, spec)
    concat_in = [
        jax.device_put(
            np.concatenate([np.asarray(in_maps[c][k]) for c in range(n_cores)], axis=0),
            sh,
        )
        for k in in_names
    ]
    concat_zeros = [
        jax.device_put(
            np.zeros((n_cores * z.shape[0], *z.shape[1:]), z.dtype), sh
        )
        for z in zero_outs
    ]
    jax.block_until_ready(concat_in)
    jax.block_until_ready(concat_zeros)
    out_arrs = sharded(*concat_in, *concat_zeros)
    return [
        {
            name: np.asarray(out_arrs[i]).reshape(n_cores, *out_avals[i].shape)[c]
            for i, name in enumerate(out_names)
        }
        for c in range(n_cores)
    ]


def _in_maps(x: np.ndarray, dc_logit: np.ndarray):
    import ml_dtypes

    wu2 = _host_weights(np.asarray(dc_logit))  # [C, RW]
    cpk = np.empty((P, NTILES * RW), np.float32)
    for t in range(NTILES):
        cpk[:, t * RW : (t + 1) * RW] = wu2[(t % 2) * P : (t % 2 + 1) * P]
    ident = np.eye(P, dtype=np.float32).astype(ml_dtypes.bfloat16)
    xr = np.ascontiguousarray(x).reshape(B * C, N).astype(ml_dtypes.bfloat16)
    return [
        {"x": xr[i * ROWS : (i + 1) * ROWS], "cpk": cpk, "ident": ident}
        for i in range(NCORES)
    ]


def kernel(x: np.ndarray, dc_logit: np.ndarray) -> np.ndarray:
    import time

    nc = _build()
    in_maps = _in_maps(x, dc_logit)
    last_err = None
    for attempt in range(3):
        try:
            results = _run_pjrt(nc, in_maps)
            break
        except Exception as e:  # transient device errors (wedged core etc.)
            last_err = e
            time.sleep(15)
    else:
        raise last_err
    outs = []
    for i in range(NCORES):
        o = results[i]["out"]  # [P, NTILES]; col t, row p -> global row t*128+p
        outs.append(o.T.reshape(BS, C))
    return np.concatenate(outs, axis=0).astype(np.float32)
